# revision 41
# baseline (speedup 1.0000x reference)
"""Trainium2 Bass kernel for nn_MemLayer (retrieval_knn).

Math:  out[b,o] = -mean_d (x[b,d] - w[o,d])^2 + bias[o]
              =  s * (x' @ w'.T)[b,o]  -  ||x_b||^2/D  +  (bias[o] - ||w_o||^2/D)

  with x' = 16*x, w' = 4096*w in fp8e4m3 and s = 2/(D*16*4096) applied on the
  ACT engine at PSUM eviction (both scale factors keep the fp8 operands inside
  the e4m3 normal range; accumulation is fp32 in PSUM).

Strategy:
  - Data-parallel shard x along batch across 8 NeuronCores (1024 rows each),
    replicate weights. No cross-core communication; gather outputs on host.
  - Per core: fp8 GEMM [1024,1024] @ [1024,4096] using DoubleRow perf mode
    (2 fp8 weights per PE cell -> contraction 256 per matmul, 256 matmuls).
  - Schedule: n-tile outer; within an n-tile the contraction (kd) loop is
    OUTER across two half-passes of 4 PSUM banks each, so the PE accumulates
    into one half while the ACT/DVE eviction chain drains the other.  The
    warm-up allocates all 8 PSUM bufs once so real halves stay aligned to
    banks 0-3/4-7 (a misaligned rotation costs ~430ns at every nt boundary).
  - Output is bf16 (halves the dominant output-DMA traffic vs fp32; ~8e-3
    absolute rounding vs a 2e-2-of-max correctness budget; fp16 is NOT used
    because the ACT fp16-convert path is ~18% slower and stalls the PE).
    Corrections fused into PSUM eviction:
      * ACT:  out_sb = bf16(psum * s + xsq[p])   (per-partition bias)
      * DVE:  out_sb += v[o]  in bf16            (v = bias - ||w||^2/D)
    then the bf16 tile DMAs straight to DRAM; host upcasts to fp32.
  - DMA feed order is tuned so the PE never waits: sync ring carries
    wk(nt0,kc0) then the kc0 x-chunk split in two separately-gated pieces
    (mt0-3 / mt4-7), then the rest of nt0's weight chunks; the scalar ring
    (whose first slot is burned by the framework ACT table load) carries
    the later x-chunks.  Remaining weight n-chunks prefetch from inside the
    nt loop, 3 tiles ahead.
"""

import numpy as np
import ml_dtypes

B, D, O = 8192, 1024, 4096
NCORES = 8
BL = B // NCORES     # 1024 rows per core
P = 128
MT = BL // P         # 8 m-tiles
NTILE = 512          # one PSUM bank of fp32
NT = O // NTILE      # 8 n-tiles

KD = D // (2 * P)    # 4 double-k-tiles (fp8 DoubleRow path)
XSCALE = 16.0        # x -> fp8 pre-scale
WSCALE = 4096.0      # w -> fp8 pre-scale

_CACHE = {}


def _get_nc():
    key = "nc_v20"
    if key in _CACHE:
        return _CACHE[key]

    import concourse.bacc as bacc
    import concourse.tile as tile
    from concourse import mybir

    nc = bacc.Bacc("TRN2", target_bir_lowering=False)

    f32 = mybir.dt.float32
    bf16 = mybir.dt.bfloat16
    mm_dt = mybir.dt.float8e4

    # kc0 of x split into two separately-gated DMAs so the first matmuls
    # are unblocked as early as possible.
    xk0a_d = nc.dram_tensor("xk0a", [P, 2, BL // 2], mm_dt, kind="ExternalInput")
    xk0b_d = nc.dram_tensor("xk0b", [P, 2, BL // 2], mm_dt, kind="ExternalInput")
    xkr_d = nc.dram_tensor("xkr", [P, KD - 1, 2, BL], mm_dt, kind="ExternalInput")
    wk_d = nc.dram_tensor("wk", [NT, P, KD, 2, NTILE], mm_dt,
                          kind="ExternalInput")
    xsq_d = nc.dram_tensor("xsq", [P, MT], f32, kind="ExternalInput")
    v_d = nc.dram_tensor("v", [1, O], bf16, kind="ExternalInput")
    out_d = nc.dram_tensor("out", [P, MT, O], bf16, kind="ExternalOutput")

    act_scale = float(2.0 / (D * XSCALE * WSCALE))
    kiters = KD

    with tile.TileContext(nc) as tc:
        with (
            tc.tile_pool(name="const", bufs=1) as cpool,
            tc.tile_pool(name="psum", bufs=8, space="PSUM") as ppool,
            tc.tile_pool(name="outp", bufs=5) as opool,
        ):
            xk_sb = cpool.tile([P, KD, 2, BL], mm_dt)
            wk_sb = cpool.tile([P, NT, KD, 2, NTILE], mm_dt)
            xsq_sb = cpool.tile([P, MT], f32)
            vb_sb = cpool.tile([P, O], bf16)

            # Warm-up: the PE HAM clock gate needs a few us of sustained
            # high-duty matmul activity to unthrottle 1.2 -> 2.4 GHz, and the
            # PE is idle anyway while the first input chunks DMA in.  A few
            # short matmuls run immediately (zk memset on gpsimd finishes
            # ~200ns after the preamble), then long 512-column streams keep
            # the duty near 100%.  Exactly 8 PSUM tiles are consumed so the
            # pool rotation stays half-aligned for the real tiles.
            zk = cpool.tile([P, 2, 128], mm_dt)
            wz = cpool.tile([P, 2, NTILE], mm_dt)
            nc.gpsimd.memset(zk[:], 0.0)
            nc.vector.memset(wz[:], 0.0)
            # Short warm-ups burn PSUM banks 0-5, long ones alternate banks
            # 6/7 — so the first REAL psum tile (bank 0) only depends on an
            # early short warm-up, and the first real matmul is gated by its
            # input DMA, not by the warm-up queue.
            warm = [ppool.tile([P, NTILE], f32, tag="ps", name=f"warm{i}")
                    for i in range(8)]
            # Full 128-column stationary so the warm-ups light the whole
            # PE array — a half-width stationary presents only ~50% duty to
            # the HAM governor and ramps the clock late/erratically.
            for i in range(12):
                nc.tensor.matmul(
                    warm[i % 6][:, :64],
                    lhsT=zk[:],
                    rhs=zk[:, :, 0:64],
                    start=True,
                    stop=True,
                    perf_mode=mybir.MatmulPerfMode.DoubleRow,
                )
            for i in range(5):
                nc.tensor.matmul(
                    warm[6 + (i % 2)][:],
                    lhsT=zk[:],
                    rhs=wz[:],
                    start=True,
                    stop=True,
                    perf_mode=mybir.MatmulPerfMode.DoubleRow,
                )

            nc.sync.dma_start(out=wk_sb[:, 0, 0, :, :], in_=wk_d[0, :, 0])
            nc.sync.dma_start(out=xk_sb[:, 0, :, 0:BL // 2], in_=xk0a_d[:])
            nc.sync.dma_start(out=xk_sb[:, 0, :, BL // 2:BL], in_=xk0b_d[:])
            # xsq first on the scalar ring: it is tiny and gates the first
            # ACT eviction (and through bank reuse, the PE) if late.
            nc.scalar.dma_start(out=xsq_sb[:], in_=xsq_d[:])
            for kc in range(1, kiters):
                nc.scalar.dma_start(out=xk_sb[:, kc, :, :],
                                    in_=xkr_d[:, kc - 1])
                nc.sync.dma_start(out=wk_sb[:, 0, kc, :, :],
                                  in_=wk_d[0, :, kc])
            # vb rides the scalar ring AFTER the x chunks: the PE must never
            # wait behind this 1MB broadcast (the DVE adds it consumes can
            # lag ~5 half-passes on the opool buffers without stalling
            # anything), while wk1 on the sync ring gates nt1's matmuls.
            nc.scalar.dma_start(out=vb_sb[:], in_=v_d[:].to_broadcast([P, O]))
            nc.sync.dma_start(out=wk_sb[:, 1], in_=wk_d[1])
            nc.sync.dma_start(out=wk_sb[:, 2], in_=wk_d[2])

            def evict(mt, ns, ob_ap, ps):
                nc.scalar.activation(
                    ob_ap,
                    ps[:],
                    mybir.ActivationFunctionType.Identity,
                    bias=xsq_sb[:, mt:mt + 1],
                    scale=act_scale,
                )
                nc.vector.tensor_add(ob_ap, ob_ap, vb_sb[:, ns])

            for nt in range(NT):
                if nt + 3 < NT:
                    nc.sync.dma_start(out=wk_sb[:, nt + 3], in_=wk_d[nt + 3])
                ns = slice(nt * NTILE, (nt + 1) * NTILE)
                # Group the 8 m-tiles; the final n-tile tapers 4/2/1/1 so the
                # after-last-matmul tail is a single-tile ACT->DVE->DMA chain.
                groups = ([range(0, 4), range(4, 6), range(6, 7), range(7, 8)]
                          if nt == NT - 1 else
                          [range(0, 4), range(4, 8)])
                for mts in groups:
                    if nt == NT - 1 and mts.start == MT - 1:
                        # Very last tile: accumulate the two 256-column
                        # halves in SEPARATE PSUM banks, first half's kc
                        # chain first — its ACT->DVE->DMA starts 4 matmuls
                        # before the last one retires, and the two half
                        # chains drain on different HWDGE rings.  This pulls
                        # in the last output-DMA completion, which gates the
                        # fixed end-of-kernel semaphore-reset storm.
                        mt = MT - 1
                        h = NTILE // 2
                        for j, eng in ((0, nc.sync), (1, nc.scalar)):
                            psh = ppool.tile([P, h], f32, tag="ps",
                                             name=f"ps_last{j}")
                            for kc in range(kiters):
                                nc.tensor.matmul(
                                    psh[:],
                                    lhsT=xk_sb[:, kc, :, mt * P:(mt + 1) * P],
                                    rhs=wk_sb[:, nt, kc, :,
                                              j * h:(j + 1) * h],
                                    start=(kc == 0),
                                    stop=(kc == kiters - 1),
                                    perf_mode=mybir.MatmulPerfMode.DoubleRow,
                                )
                            nsj = slice(ns.start + j * h,
                                        ns.start + (j + 1) * h)
                            obh = opool.tile([P, h], bf16, tag="obs",
                                             name=f"obs_last{j}")
                            evict(mt, nsj, obh[:], psh)
                            eng.dma_start(out=out_d[:, mt, nsj], in_=obh[:])
                        continue
                    pss = {}
                    for mt in mts:
                        pss[mt] = ppool.tile([P, NTILE], f32, tag="ps",
                                             name=f"ps{nt}_{mt}")
                    for kc in range(kiters):
                        for mt in mts:
                            nc.tensor.matmul(
                                pss[mt][:],
                                lhsT=xk_sb[:, kc, :, mt * P:(mt + 1) * P],
                                rhs=wk_sb[:, nt, kc, :, :],
                                start=(kc == 0),
                                stop=(kc == kiters - 1),
                                perf_mode=mybir.MatmulPerfMode.DoubleRow,
                            )
                    if nt == NT - 1:
                        for mt in mts:
                            obs = opool.tile([P, NTILE], bf16, tag="obs")
                            evict(mt, ns, obs[:], pss[mt])
                            nc.sync.dma_start(out=out_d[:, mt, ns],
                                              in_=obs[:])
                    else:
                        ob = opool.tile([P, len(mts), NTILE], bf16)
                        for j, mt in enumerate(mts):
                            evict(mt, ns, ob[:, j, :], pss[mt])
                        mt0 = mts[0]
                        nc.sync.dma_start(
                            out=out_d[:, mt0:mt0 + len(mts), ns], in_=ob[:])

            # Post-work: ~10 dummy matmuls overlap the final eviction/DMA
            # tail on the otherwise-idle PE so the HAM governor does not
            # down-clock (k=8 -> k=4) before the end-of-kernel drain barrier
            # runs — the barrier's cross-engine event hops dominate the tail.
            cool = ppool.tile([P, NTILE], f32, tag="ps", name="cool")
            for _ in range(10):
                nc.tensor.matmul(
                    cool[:],
                    lhsT=zk[:, :, :],
                    rhs=wz[:],
                    start=True,
                    stop=True,
                    perf_mode=mybir.MatmulPerfMode.DoubleRow,
                )

    nc.finalize()
    _CACHE[key] = nc
    return nc


def _prep_inputs(x, weights, bias):
    """Shard + lay out host inputs -> per-core in_maps."""
    x = np.asarray(x, dtype=np.float32)
    weights = np.asarray(weights, dtype=np.float32)
    bias = np.asarray(bias, dtype=np.float32)

    w_sq = np.einsum("od,od->o", weights, weights)
    v = np.ascontiguousarray(
        (bias - w_sq / np.float32(D)).reshape(1, O).astype(ml_dtypes.bfloat16)
    )

    dt = ml_dtypes.float8_e4m3
    # k = kd*256 + i*128 + p
    wT = weights.T * np.float32(WSCALE)                   # [D, O]
    wk = np.ascontiguousarray(
        wT.reshape(KD, 2, P, NT, NTILE)
        .transpose(3, 2, 0, 1, 4)
        .astype(dt)
    )

    in_maps = []
    for c in range(NCORES):
        xs = x[c * BL:(c + 1) * BL]                            # [BL, D] fp32
        xT = xs.T                                              # [D, BL]
        xk = (xT.reshape(KD, 2, P, BL) * np.float32(XSCALE)) \
            .transpose(2, 0, 1, 3).astype(dt)                  # [P, KD, 2, BL]
        xsq = -np.einsum("bd,bd->b", xs, xs) / np.float32(D)   # [BL]
        xsq_l = np.ascontiguousarray(xsq.reshape(MT, P).T)     # [P, MT]
        in_maps.append({
            "xk0a": np.ascontiguousarray(xk[:, 0, :, :BL // 2]),
            "xk0b": np.ascontiguousarray(xk[:, 0, :, BL // 2:]),
            "xkr": np.ascontiguousarray(xk[:, 1:]),
            "wk": wk,
            "xsq": xsq_l,
            "v": v,
        })
    return in_maps


def _gather(results):
    parts = []
    for c in range(NCORES):
        o = results[c]["out"]                                  # [P, MT, O]
        parts.append(o.transpose(1, 0, 2).reshape(BL, O))
    return np.concatenate(parts, axis=0).astype(np.float32)


def _run(in_maps, **kwargs):
    from concourse.bass_utils import run_bass_kernel_spmd

    nc = _get_nc()
    return run_bass_kernel_spmd(nc, in_maps, core_ids=list(range(NCORES)), **kwargs)


def kernel(x, weights, bias):
    in_maps = _prep_inputs(x, weights, bias)
    res = _run(in_maps)
    return _gather(res.results)


# revision 42
# speedup vs baseline: 1.1746x; 1.1746x over previous
"""Trainium2 Bass kernel for nn_MemLayer (retrieval_knn).

Math:  out[b,o] = -mean_d (x[b,d] - w[o,d])^2 + bias[o]
              =  s * (x' @ w'.T)[b,o]  -  ||x_b||^2/D  +  (bias[o] - ||w_o||^2/D)

  with x' = 16*x, w' = 4096*w in fp8e4m3 and s = 2/(D*16*4096) applied on the
  ACT engine at PSUM eviction (both scale factors keep the fp8 operands inside
  the e4m3 normal range; accumulation is fp32 in PSUM).

Strategy:
  - Data-parallel shard x along batch across 8 NeuronCores (1024 rows each),
    replicate weights. No cross-core communication; gather outputs on host.
  - Per core: fp8 GEMM [1024,1024] @ [1024,4096] using DoubleRow perf mode
    (2 fp8 weights per PE cell -> contraction 256 per matmul, 256 matmuls).
  - Schedule: n-tile outer; within an n-tile the contraction (kd) loop is
    OUTER across two half-passes of 4 PSUM banks each, so the PE accumulates
    into one half while the ACT/DVE eviction chain drains the other.  The
    warm-up allocates all 8 PSUM bufs once so real halves stay aligned to
    banks 0-3/4-7 (a misaligned rotation costs ~430ns at every nt boundary).
  - Output is bf16 (halves the dominant output-DMA traffic vs fp32; ~8e-3
    absolute rounding vs a 2e-2-of-max correctness budget; fp16 is NOT used
    because the ACT fp16-convert path is ~18% slower and stalls the PE).
    Corrections fused into PSUM eviction:
      * ACT:  out_sb = bf16(psum * s + xsq[p])   (per-partition bias)
      * DVE:  out_sb += v[o]  in bf16            (v = bias - ||w||^2/D)
    then the bf16 tile DMAs straight to DRAM; host upcasts to fp32.
  - DMA feed order is tuned so the PE never waits: sync ring carries
    wk(nt0,kc0) then the kc0 x-chunk split in two separately-gated pieces
    (mt0-3 / mt4-7), then the rest of nt0's weight chunks; the scalar ring
    (whose first slot is burned by the framework ACT table load) carries
    the later x-chunks.  Remaining weight n-chunks prefetch from inside the
    nt loop, 3 tiles ahead.
"""

import numpy as np
import ml_dtypes

B, D, O = 8192, 1024, 4096
NCORES = 8
BL = B // NCORES     # 1024 rows per core
P = 128
MT = BL // P         # 8 m-tiles
NTILE = 512          # one PSUM bank of fp32
NT = O // NTILE      # 8 n-tiles

KD = D // (2 * P)    # 4 double-k-tiles (fp8 DoubleRow path)
XSCALE = 16.0        # x -> fp8 pre-scale
WSCALE = 4096.0      # w -> fp8 pre-scale

_CACHE = {}


def _get_nc():
    key = "nc_v19_final"
    if key in _CACHE:
        return _CACHE[key]

    import concourse.bacc as bacc
    import concourse.tile as tile
    from concourse import mybir

    nc = bacc.Bacc("TRN2", target_bir_lowering=False)

    f32 = mybir.dt.float32
    bf16 = mybir.dt.bfloat16
    mm_dt = mybir.dt.float8e4

    # kc0 of x split into two separately-gated DMAs so the first matmuls
    # are unblocked as early as possible.
    xk0a_d = nc.dram_tensor("xk0a", [P, 2, BL // 2], mm_dt, kind="ExternalInput")
    xk0b_d = nc.dram_tensor("xk0b", [P, 2, BL // 2], mm_dt, kind="ExternalInput")
    xkr_d = nc.dram_tensor("xkr", [P, KD - 1, 2, BL], mm_dt, kind="ExternalInput")
    wk_d = nc.dram_tensor("wk", [NT, P, KD, 2, NTILE], mm_dt,
                          kind="ExternalInput")
    xsq_d = nc.dram_tensor("xsq", [P, MT], f32, kind="ExternalInput")
    v_d = nc.dram_tensor("v", [1, O], bf16, kind="ExternalInput")
    out_d = nc.dram_tensor("out", [P, MT, O], bf16, kind="ExternalOutput")

    act_scale = float(2.0 / (D * XSCALE * WSCALE))
    kiters = KD

    with tile.TileContext(nc) as tc:
        with (
            tc.tile_pool(name="const", bufs=1) as cpool,
            tc.tile_pool(name="psum", bufs=8, space="PSUM") as ppool,
            tc.tile_pool(name="outp", bufs=5) as opool,
        ):
            xk_sb = cpool.tile([P, KD, 2, BL], mm_dt)
            wk_sb = cpool.tile([P, NT, KD, 2, NTILE], mm_dt)
            xsq_sb = cpool.tile([P, MT], f32)
            vb_sb = cpool.tile([P, O], bf16)

            # Warm-up: the PE HAM clock gate needs a few us of sustained
            # high-duty matmul activity to unthrottle 1.2 -> 2.4 GHz, and the
            # PE is idle anyway while the first input chunks DMA in.  A few
            # short matmuls run immediately (zk memset on gpsimd finishes
            # ~200ns after the preamble), then long 512-column streams keep
            # the duty near 100%.  Exactly 8 PSUM tiles are consumed so the
            # pool rotation stays half-aligned for the real tiles.
            zk = cpool.tile([P, 2, 128], mm_dt)
            wz = cpool.tile([P, 2, NTILE], mm_dt)
            nc.gpsimd.memset(zk[:], 0.0)
            nc.vector.memset(wz[:], 0.0)
            # Short warm-ups burn PSUM banks 0-5, long ones alternate banks
            # 6/7 — so the first REAL psum tile (bank 0) only depends on an
            # early short warm-up, and the first real matmul is gated by its
            # input DMA, not by the warm-up queue.
            warm = [ppool.tile([P, NTILE], f32, tag="ps", name=f"warm{i}")
                    for i in range(8)]
            # Full 128-column stationary so the warm-ups light the whole
            # PE array — a half-width stationary presents only ~50% duty to
            # the HAM governor and ramps the clock late/erratically.
            for i in range(6):
                nc.tensor.matmul(
                    warm[i][:, :64],
                    lhsT=zk[:],
                    rhs=zk[:, :, 0:64],
                    start=True,
                    stop=True,
                    perf_mode=mybir.MatmulPerfMode.DoubleRow,
                )
            for i in range(5):
                nc.tensor.matmul(
                    warm[6 + (i % 2)][:],
                    lhsT=zk[:],
                    rhs=wz[:],
                    start=True,
                    stop=True,
                    perf_mode=mybir.MatmulPerfMode.DoubleRow,
                )

            nc.sync.dma_start(out=wk_sb[:, 0, 0, :, :], in_=wk_d[0, :, 0])
            nc.sync.dma_start(out=xk_sb[:, 0, :, 0:BL // 2], in_=xk0a_d[:])
            nc.sync.dma_start(out=xk_sb[:, 0, :, BL // 2:BL], in_=xk0b_d[:])
            # xsq first on the scalar ring: it is tiny and gates the first
            # ACT eviction (and through bank reuse, the PE) if late.
            nc.scalar.dma_start(out=xsq_sb[:], in_=xsq_d[:])
            for kc in range(1, kiters):
                nc.scalar.dma_start(out=xk_sb[:, kc, :, :],
                                    in_=xkr_d[:, kc - 1])
                nc.sync.dma_start(out=wk_sb[:, 0, kc, :, :],
                                  in_=wk_d[0, :, kc])
            # vb rides the scalar ring AFTER the x chunks: the PE must never
            # wait behind this 1MB broadcast (the DVE adds it consumes can
            # lag ~5 half-passes on the opool buffers without stalling
            # anything), while wk1 on the sync ring gates nt1's matmuls.
            nc.scalar.dma_start(out=vb_sb[:], in_=v_d[:].to_broadcast([P, O]))
            nc.sync.dma_start(out=wk_sb[:, 1], in_=wk_d[1])
            nc.sync.dma_start(out=wk_sb[:, 2], in_=wk_d[2])

            def evict(mt, ns, ob_ap, ps):
                nc.scalar.activation(
                    ob_ap,
                    ps[:],
                    mybir.ActivationFunctionType.Identity,
                    bias=xsq_sb[:, mt:mt + 1],
                    scale=act_scale,
                )
                nc.vector.tensor_add(ob_ap, ob_ap, vb_sb[:, ns])

            for nt in range(NT):
                if nt + 3 < NT:
                    nc.sync.dma_start(out=wk_sb[:, nt + 3], in_=wk_d[nt + 3])
                ns = slice(nt * NTILE, (nt + 1) * NTILE)
                # Group the 8 m-tiles; the final n-tile tapers 4/2/1/1 so the
                # after-last-matmul tail is a single-tile ACT->DVE->DMA chain.
                groups = ([range(0, 4), range(4, 6), range(6, 7), range(7, 8)]
                          if nt == NT - 1 else
                          [range(0, 4), range(4, 8)])
                for mts in groups:
                    if nt == NT - 1 and mts.start == MT - 1:
                        # Very last tile: accumulate the two 256-column
                        # halves in SEPARATE PSUM banks, first half's kc
                        # chain first — its ACT->DVE->DMA starts 4 matmuls
                        # before the last one retires, and the two half
                        # chains drain on different HWDGE rings.  This pulls
                        # in the last output-DMA completion, which gates the
                        # fixed end-of-kernel semaphore-reset storm.
                        mt = MT - 1
                        h = NTILE // 2
                        for j, eng in ((0, nc.sync), (1, nc.scalar)):
                            psh = ppool.tile([P, h], f32, tag="ps",
                                             name=f"ps_last{j}")
                            for kc in range(kiters):
                                nc.tensor.matmul(
                                    psh[:],
                                    lhsT=xk_sb[:, kc, :, mt * P:(mt + 1) * P],
                                    rhs=wk_sb[:, nt, kc, :,
                                              j * h:(j + 1) * h],
                                    start=(kc == 0),
                                    stop=(kc == kiters - 1),
                                    perf_mode=mybir.MatmulPerfMode.DoubleRow,
                                )
                            nsj = slice(ns.start + j * h,
                                        ns.start + (j + 1) * h)
                            obh = opool.tile([P, h], bf16, tag="obs",
                                             name=f"obs_last{j}")
                            evict(mt, nsj, obh[:], psh)
                            eng.dma_start(out=out_d[:, mt, nsj], in_=obh[:])
                        continue
                    pss = {}
                    for mt in mts:
                        pss[mt] = ppool.tile([P, NTILE], f32, tag="ps",
                                             name=f"ps{nt}_{mt}")
                    for kc in range(kiters):
                        for mt in mts:
                            nc.tensor.matmul(
                                pss[mt][:],
                                lhsT=xk_sb[:, kc, :, mt * P:(mt + 1) * P],
                                rhs=wk_sb[:, nt, kc, :, :],
                                start=(kc == 0),
                                stop=(kc == kiters - 1),
                                perf_mode=mybir.MatmulPerfMode.DoubleRow,
                            )
                    if nt == NT - 1:
                        for mt in mts:
                            obs = opool.tile([P, NTILE], bf16, tag="obs")
                            evict(mt, ns, obs[:], pss[mt])
                            nc.sync.dma_start(out=out_d[:, mt, ns],
                                              in_=obs[:])
                    else:
                        ob = opool.tile([P, len(mts), NTILE], bf16)
                        for j, mt in enumerate(mts):
                            evict(mt, ns, ob[:, j, :], pss[mt])
                        mt0 = mts[0]
                        nc.sync.dma_start(
                            out=out_d[:, mt0:mt0 + len(mts), ns], in_=ob[:])

            # Post-work: ~10 dummy matmuls overlap the final eviction/DMA
            # tail on the otherwise-idle PE so the HAM governor does not
            # down-clock (k=8 -> k=4) before the end-of-kernel drain barrier
            # runs — the barrier's cross-engine event hops dominate the tail.
            cool = ppool.tile([P, NTILE], f32, tag="ps", name="cool")
            for _ in range(10):
                nc.tensor.matmul(
                    cool[:],
                    lhsT=zk[:, :, :],
                    rhs=wz[:],
                    start=True,
                    stop=True,
                    perf_mode=mybir.MatmulPerfMode.DoubleRow,
                )

    nc.finalize()
    _CACHE[key] = nc
    return nc


def _prep_inputs(x, weights, bias):
    """Shard + lay out host inputs -> per-core in_maps."""
    x = np.asarray(x, dtype=np.float32)
    weights = np.asarray(weights, dtype=np.float32)
    bias = np.asarray(bias, dtype=np.float32)

    w_sq = np.einsum("od,od->o", weights, weights)
    v = np.ascontiguousarray(
        (bias - w_sq / np.float32(D)).reshape(1, O).astype(ml_dtypes.bfloat16)
    )

    dt = ml_dtypes.float8_e4m3
    # k = kd*256 + i*128 + p
    wT = weights.T * np.float32(WSCALE)                   # [D, O]
    wk = np.ascontiguousarray(
        wT.reshape(KD, 2, P, NT, NTILE)
        .transpose(3, 2, 0, 1, 4)
        .astype(dt)
    )

    in_maps = []
    for c in range(NCORES):
        xs = x[c * BL:(c + 1) * BL]                            # [BL, D] fp32
        xT = xs.T                                              # [D, BL]
        xk = (xT.reshape(KD, 2, P, BL) * np.float32(XSCALE)) \
            .transpose(2, 0, 1, 3).astype(dt)                  # [P, KD, 2, BL]
        xsq = -np.einsum("bd,bd->b", xs, xs) / np.float32(D)   # [BL]
        xsq_l = np.ascontiguousarray(xsq.reshape(MT, P).T)     # [P, MT]
        in_maps.append({
            "xk0a": np.ascontiguousarray(xk[:, 0, :, :BL // 2]),
            "xk0b": np.ascontiguousarray(xk[:, 0, :, BL // 2:]),
            "xkr": np.ascontiguousarray(xk[:, 1:]),
            "wk": wk,
            "xsq": xsq_l,
            "v": v,
        })
    return in_maps


def _gather(results):
    parts = []
    for c in range(NCORES):
        o = results[c]["out"]                                  # [P, MT, O]
        parts.append(o.transpose(1, 0, 2).reshape(BL, O))
    return np.concatenate(parts, axis=0).astype(np.float32)


def _run(in_maps, **kwargs):
    from concourse.bass_utils import run_bass_kernel_spmd

    nc = _get_nc()
    return run_bass_kernel_spmd(nc, in_maps, core_ids=list(range(NCORES)), **kwargs)


def kernel(x, weights, bias):
    in_maps = _prep_inputs(x, weights, bias)
    res = _run(in_maps)
    return _gather(res.results)
